# revision 6
# baseline (speedup 1.0000x reference)
"""Trainium2 Bass kernel for nn_Decoder sparse-attention decode step (v3).

Math (algebraically reduced from the reference):
    phi1 = output[prev_node] @ W1.T + b1                      # [HID]
    u    = (phi1 @ W2) / sqrt(DH)                             # [H]
    cst  = (phi1 @ b2) / sqrt(DH)                             # scalar
    s[n]    = u . output[n] + cst                             # [N]
    attn[n] = 10 * adj[n] * tanh(s[n])
    w = softmax(attn); w *= (attn != 0); p = w.max(); sel = argmax(w)

v3 strategy (throughput-oriented; DMA is the roofline):
  * Host drops all adj==0 rows (~50% of N): their attn is exactly 0, so
    only their count matters for the softmax denominator (added back on
    host as count0 * exp(-10) with fixed shift 10).
  * Compacted rows are quantized to fp8-e4m3 and staged PRE-TRANSPOSED
    as [64 partitions, 2, rows] (feature h = i*64 + k), so the device
    does zero transposes: one DoubleRow fp8 matmul per 512-row group
    accumulates scores straight into PSUM, group g on psum partition g
    (one-hot weight column windows).
  * Device returns per-group (z_partial, raw-score max) only.  The
    argmax itself is done EXACTLY on the host: the top groups by
    quantized max (within a margin) are re-scored in fp64 against the
    original fp32 rows, so fp8 noise cannot flip the selection.
  * z comes from the device: exp(10*tanh(s)-10) accumulated per group
    (ACT accum), pad rows contribute exp(-20) each (x_pad = -64*sign(u)
    saturates tanh to -1.0) and are subtracted on host.

Per core: 25 groups x 512 rows = 12800 rows (fp8: 1.64 MB -> ~4.6us at
~360 GB/s/core).  PE: 25 DoubleRow matmuls ~3us.  Epilogue: 2 banks
(13+12 groups), tanh+exp on ACT, raw max on DVE, [25,2] stats out.
"""

from contextlib import ExitStack

import numpy as np
import ml_dtypes

import concourse.bass as bass
import concourse.bacc as bacc
import concourse.tile as tile
from concourse import mybir

F32 = mybir.dt.float32
F8 = mybir.dt.float8e4
E4NP = ml_dtypes.float8_e4m3

N = 200000
H = 128
HID = 512
DH = 512.0
CLIP = 10.0
NCORES = 8
GROUP = 512
NGRP = 25                      # groups per core
RC = NGRP * GROUP              # 12800 padded rows per core
NBANK0 = 13                    # groups 0..12 -> psum bank A
SHIFT = 10.0                   # fixed softmax shift (attn <= 10 always)
MARGIN = 0.5                   # host recheck margin, in s units
CHG = 5                        # groups per DMA chunk
NCH = NGRP // CHG


def build_program_v3(reps=1, mode="full"):
    """mode: 'full' | 'dmaonly' (only the x DMAs per rep) |
    'nodma' (DMA once, repeat compute on stale tiles)."""
    nc = bacc.Bacc(
        "TRN2", target_bir_lowering=False, debug=False, num_devices=NCORES
    )

    x_d = nc.dram_tensor("x8", [64, NGRP * 1024], F8, kind="ExternalInput").ap()
    uw_d = nc.dram_tensor("uw", [64, NGRP * 64], F8, kind="ExternalInput").ap()
    cs_d = nc.dram_tensor("cs", [32, 4], F32, kind="ExternalInput").ap()
    out_d = nc.dram_tensor("o", [NGRP, 2], F32, kind="ExternalOutput").ap()

    with tile.TileContext(nc) as tc, ExitStack() as ctx:
        const = ctx.enter_context(tc.tile_pool(name="const", bufs=1))
        xp = ctx.enter_context(tc.tile_pool(name="xp", bufs=3))
        pp = ctx.enter_context(tc.tile_pool(name="pp", bufs=2, space="PSUM"))
        sm = ctx.enter_context(tc.tile_pool(name="sm", bufs=2))

        uwt = const.tile([64, NGRP * 64], F8)
        nc.sync.dma_start(uwt, uw_d)
        uwv = uwt.rearrange("p (g i m) -> p g i m", g=NGRP, i=2)
        cs = const.tile([32, 4], F32)
        nc.sync.dma_start(cs, cs_d)

        xts_fixed = None
        if mode == "nodma":
            xn = ctx.enter_context(tc.tile_pool(name="xn", bufs=1))
            xts_fixed = []
            for c in range(NCH):
                xt = xn.tile([64, CHG * 1024], F8, tag=f"xtf{c}")
                nc.sync.dma_start(xt, x_d[:, c * CHG * 1024:(c + 1) * CHG * 1024])
                xts_fixed.append(xt)

        for _rep in range(reps):
            if mode == "dmaonly":
                for c in range(NCH):
                    xt = xp.tile([64, CHG * 1024], F8, tag="xt")
                    nc.sync.dma_start(
                        xt, x_d[:, c * CHG * 1024:(c + 1) * CHG * 1024]
                    )
                continue

            pA = pp.tile([32, 512], F32, tag="pA")
            pB = pp.tile([32, 512], F32, tag="pB")

            for c in range(NCH):
                if xts_fixed is not None:
                    xt = xts_fixed[c]
                else:
                    xt = xp.tile([64, CHG * 1024], F8, tag="xt")
                    nc.sync.dma_start(
                        xt, x_d[:, c * CHG * 1024:(c + 1) * CHG * 1024]
                    )
                xv = xt.rearrange("p (g i r) -> p g i r", g=CHG, i=2)
                for j in range(CHG):
                    g = c * CHG + j
                    bank = pA if g < NBANK0 else pB
                    nc.tensor.matmul(
                        bank,
                        uwv[:, g],
                        xv[:, j],
                        start=(g == 0 or g == NBANK0),
                        stop=(g == NBANK0 - 1 or g == NGRP - 1),
                        perf_mode=mybir.MatmulPerfMode.DoubleRow,
                    )

            NB1 = NGRP - NBANK0
            tA = sm.tile([NBANK0, 512], F32, tag="tA")
            eA = sm.tile([NBANK0, 512], F32, tag="eA")
            finA = sm.tile([NBANK0, 2], F32, tag="finA")
            tB = sm.tile([NB1, 512], F32, tag="tB")
            eB = sm.tile([NB1, 512], F32, tag="eB")
            finB = sm.tile([NB1, 2], F32, tag="finB")

            # bank A: groups 0..12 on psum partitions 0..12
            nc.vector.tensor_reduce(
                finA[:, 1:2], pA[0:NBANK0, :],
                axis=mybir.AxisListType.X, op=mybir.AluOpType.max,
            )
            nc.scalar.activation(
                tA, pA[0:NBANK0, :],
                mybir.ActivationFunctionType.Tanh,
                bias=cs[0:NBANK0, 0:1], scale=cs[0:NBANK0, 1:2],
            )
            nc.scalar.activation(
                eA, tA,
                mybir.ActivationFunctionType.Exp,
                bias=cs[0:NBANK0, 2:3], scale=float(CLIP),
                accum_out=finA[:, 0:1],
            )
            # bank B: groups 13..24 on psum partitions 0..11
            nc.vector.tensor_reduce(
                finB[:, 1:2], pB[0:NB1, :],
                axis=mybir.AxisListType.X, op=mybir.AluOpType.max,
            )
            nc.scalar.activation(
                tB, pB[0:NB1, :],
                mybir.ActivationFunctionType.Tanh,
                bias=cs[0:NB1, 0:1], scale=cs[0:NB1, 1:2],
            )
            nc.scalar.activation(
                eB, tB,
                mybir.ActivationFunctionType.Exp,
                bias=cs[0:NB1, 2:3], scale=float(CLIP),
                accum_out=finB[:, 0:1],
            )

            nc.sync.dma_start(out_d[0:NBANK0, :], finA)
            nc.sync.dma_start(out_d[NBANK0:NGRP, :], finB)

        if mode == "dmaonly":
            zo = sm.tile([NGRP, 2], F32, tag="zo")
            nc.vector.memset(zo, 0.0)
            nc.sync.dma_start(out_d, zo)

    nc.compile()
    return nc


_CACHE = {}


def _get_program():
    if "nc3" not in _CACHE:
        _CACHE["nc3"] = build_program_v3()
    return _CACHE["nc3"]


def _prep(output, adj_modified, W1, b1, W2, b2, prev_node):
    """Host-side scalar/vector prep shared by in-map building and combine."""
    output = np.ascontiguousarray(np.asarray(output, dtype=np.float32))
    adj = np.asarray(adj_modified, dtype=np.float32)
    W1 = np.asarray(W1, dtype=np.float64)
    b1 = np.asarray(b1, dtype=np.float64)
    W2 = np.asarray(W2, dtype=np.float64)
    b2 = np.asarray(b2, dtype=np.float64)
    pn = int(np.asarray(prev_node))

    v_i = output[pn].astype(np.float64)
    phi1 = W1 @ v_i + b1
    u = (phi1 @ W2) / np.sqrt(DH)
    cst = float(phi1 @ b2) / np.sqrt(DH)

    umax = float(np.abs(u).max())
    SC = 128.0 / umax if umax > 0 else 1.0
    u_q = (u * SC).astype(np.float32).astype(E4NP)

    idx = np.nonzero(adj > 0)[0].astype(np.int64)
    return output, u, cst, SC, u_q, idx


def make_in_maps_v3(output, adj_modified, W1, b1, W2, b2, prev_node):
    output, u, cst, SC, u_q, idx = _prep(
        output, adj_modified, W1, b1, W2, b2, prev_node
    )
    T = idx.size
    TOT = RC * NCORES
    assert T <= TOT, f"capacity exceeded: {T} > {TOT}"

    u_qf = u_q.astype(np.float32)
    x_pad = (-64.0 * np.sign(u_qf)).astype(E4NP)

    xs = np.empty((TOT, H), dtype=E4NP)
    xs[:T] = output[idx].astype(E4NP)
    xs[T:] = x_pad[None, :]

    # uw[k, g*64 + i*32 + m] = u_q[i*64 + k] iff m == psum partition of
    # group g (bank A: m = g; bank B: m = g - NBANK0), else 0
    uw = np.zeros((64, NGRP, 2, 32), dtype=E4NP)
    u2 = u_q.reshape(2, 64)                    # [i, k]
    for g in range(NGRP):
        m = g if g < NBANK0 else g - NBANK0
        uw[:, g, :, m] = u2.T
    uw = np.ascontiguousarray(uw.reshape(64, NGRP * 64))

    cs = np.zeros((32, 4), dtype=np.float32)
    cs[:, 0] = np.float32(cst)
    cs[:, 1] = np.float32(1.0 / SC)
    cs[:, 2] = np.float32(-SHIFT)

    in_maps = []
    for c in range(NCORES):
        xc = xs[c * RC:(c + 1) * RC]                # [RC, 128]
        # [g, r, i, k] -> [k, g, i, r]
        x8 = np.ascontiguousarray(
            xc.reshape(NGRP, GROUP, 2, 64).transpose(3, 0, 2, 1)
        ).reshape(64, NGRP * 1024)
        in_maps.append({"x8": x8, "uw": uw, "cs": cs})
    return in_maps


def combine_v3(stats, output, u, cst, SC, idx):
    """stats: [NCORES, NGRP, 2] f32 of (z_partial, raw max).  Exact
    host-side argmax via fp64 recheck of top groups."""
    stats = np.asarray(stats, dtype=np.float64)
    T = idx.size
    TOT = RC * NCORES
    count0 = N - T
    padcount = TOT - T

    # softmax denominator; pads saturate tanh to -1 => exp(-20) each
    u_qf = (u * SC).astype(np.float32).astype(E4NP).astype(np.float64)
    x_padf = (-64.0 * np.sign(u_qf)).astype(E4NP).astype(np.float64)
    pad_t = float(np.tanh(np.float32(float(x_padf @ u_qf) / SC + cst)))
    pad_contrib = padcount * np.exp(10.0 * pad_t - SHIFT)
    z = float(stats[:, :, 0].sum()) - pad_contrib + count0 * np.exp(-SHIFT)

    # candidate groups by quantized raw max
    smax = stats[:, :, 1].reshape(-1) / SC      # without cst
    gmax = float(smax.max())
    cand = np.nonzero(smax >= gmax - MARGIN)[0]

    best_attn = -np.inf
    best_row = -1
    for cg in cand:
        glo = int(cg) * GROUP                  # global padded row offset
        ghi = glo + GROUP
        if glo >= T:
            continue                           # pure padding group
        rr = idx[glo:min(ghi, T)]              # original row ids
        xr = output[rr].astype(np.float64)
        s_ex = xr @ u + cst
        attn_ex = 10.0 * np.tanh(s_ex)
        # replicate reference's (attn != 0) mask in fp32
        attn32 = (np.float32(10.0) * np.tanh(
            (xr.astype(np.float32) @ u.astype(np.float32))
            + np.float32(cst))).astype(np.float32)
        attn_ex = np.where(attn32 == 0.0, -np.inf, attn_ex)
        mx = float(attn_ex.max())
        if not np.isfinite(mx):
            continue
        tied = rr[attn_ex == mx]
        row = int(tied.min())
        if mx > best_attn or (mx == best_attn and row < best_row):
            best_attn = mx
            best_row = row

    if best_row < 0 or z <= 0:
        return np.int32(0), np.float32(0.0)
    p = np.exp(best_attn - SHIFT) / z
    return np.int32(best_row), np.float32(p)


def kernel(output, adj_modified, W1, b1, W2, b2, prev_node):
    from concourse.bass_utils import run_bass_kernel_spmd

    outf, u, cst, SC, u_q, idx = _prep(
        output, adj_modified, W1, b1, W2, b2, prev_node
    )
    if idx.size == 0:
        return np.int32(0), np.float32(0.0)

    nc = _get_program()
    in_maps = make_in_maps_v3(
        output, adj_modified, W1, b1, W2, b2, prev_node
    )
    res = run_bass_kernel_spmd(nc, in_maps, core_ids=list(range(NCORES)))
    stats = np.stack([res.results[c]["o"] for c in range(NCORES)])
    return combine_v3(stats, outf, u, cst, SC, idx)


# revision 20
# speedup vs baseline: 1.1597x; 1.1597x over previous
"""Trainium2 Bass kernel for nn_Decoder sparse-attention decode step (v6).

Math (algebraically reduced from the reference):
    phi1 = output[prev_node] @ W1.T + b1                      # [HID]
    u    = (phi1 @ W2) / sqrt(DH)                             # [H]
    cst  = (phi1 @ b2) / sqrt(DH)                             # scalar
    s[n]    = u . output[n] + cst                             # [N]
    attn[n] = 10 * adj[n] * tanh(s[n])
    w = softmax(attn); w *= (attn != 0); p = w.max(); sel = argmax(w)

v7 strategy (throughput-oriented):
  * Host drops all adj==0 rows (~50% of N): their attn is exactly 0, so
    only their count matters for the softmax denominator (added back on
    host as count0 * exp(-10) with fixed shift 10).
  * Compacted rows are quantized to fp8-e4m3 and staged PRE-TRANSPOSED
    (feature h = i*64 + k packed for DoubleRow): zero device transposes.
    One DoubleRow fp8 matmul per 512-row group accumulates scores
    straight into PSUM; group g's one-hot weight column g places it on
    psum partition g of ONE [32,512] psum bank.
  * PE instruction-count minimization: the measured PE floor here is
    ~150 ns per instruction, so group g maps to psum bank g%4 (four
    [32,512] banks) with one-hot weight column g//4 — only 7 distinct
    weight windows.  The redundant standalone LDWEIGHTS instructions
    (same weights AP as the previous load) are deleted post-Tile:
    25 matmuls need just 7 weight loads.
  * Epilogue per bank: DVE max (raw scores, per partition), ACT tanh,
    ACT exp+accum into per-partition z (bf16 activations halve ACT
    time; only the fp32 accumulator and max feed the result).  [32,8]
    stats DMA'd out on the gpsimd (SWDGE) queue so the sync DMA queue
    never stalls.  Psum one-hot zero rows contribute the known constant
    exp(10*tanh(cst)-10) per element to z; host subtracts them.
  * The argmax is done EXACTLY on the host: the top groups by quantized
    max (within a margin) are re-scored in fp64 against the original
    fp32 rows, so fp8 noise cannot flip the selection.
  * Pad rows use x_pad = -64*sign(u): tanh saturates to -1.0 exactly,
    contributing exp(-20) each to z, subtracted on host.

Per core: 25 groups x 512 rows = 12800 padded rows (fp8: 1.64 MB ->
~3.5us at ~460 GB/s/core measured sync-queue DMA).
"""

from contextlib import ExitStack

import numpy as np
import ml_dtypes

import concourse.bass as bass
import concourse.bacc as bacc
import concourse.tile as tile
from concourse import mybir

F32 = mybir.dt.float32
F8 = mybir.dt.float8e4
E4NP = ml_dtypes.float8_e4m3

N = 200000
H = 128
HID = 512
DH = 512.0
CLIP = 10.0
NCORES = 8
GROUP = 512
NGRP = 25                      # groups per core
RC = NGRP * GROUP              # 12800 padded rows per core
NBANK = 4                      # psum banks; group g -> bank g%4
NWQ = (NGRP + NBANK - 1) // NBANK   # 7 weight windows (column g//4)
NPART = 32                     # partitions per psum bank
SHIFT = 10.0                   # fixed softmax shift (attn <= 10 always)
MARGIN = 0.5                   # host recheck margin, in s units
CHGS = (4, 4, 4, 4, 4, 4, 1)   # DMA chunks in groups, quad-aligned


def group_slot(g):
    """(bank, partition) of group g."""
    return g % NBANK, g // NBANK


def build_program_v7(reps=1, mode="full", dedup=True, act16=True):
    """mode: 'full' | 'dmaonly' (only the x DMAs per rep) |
    'nodma' (DMA once, repeat compute on stale tiles) |
    'peonly' (DMA once, repeat matmuls only) |
    'epionly' (DMA + matmuls once, repeat the epilogue only).
    dedup: delete standalone LDWEIGHTS whose weights AP repeats.
    act16: bf16 activation outputs (z accum stays fp32)."""
    nc = bacc.Bacc(
        "TRN2", target_bir_lowering=False, debug=False, num_devices=NCORES
    )

    BF16 = mybir.dt.bfloat16 if act16 else F32

    x_d = nc.dram_tensor(
        "x8", [64, NGRP * 1024], F8, kind="ExternalInput"
    ).ap()
    uw_d = nc.dram_tensor(
        "uw", [64, NWQ * 64], F8, kind="ExternalInput"
    ).ap()
    cs_d = nc.dram_tensor("cs", [NPART, 4], F32, kind="ExternalInput").ap()
    out_d = nc.dram_tensor("o", [NPART, 8], F32, kind="ExternalOutput").ap()

    chunk_off = []
    off = 0
    for w in CHGS:
        chunk_off.append((off, w))
        off += w
    assert off == NGRP

    with tile.TileContext(nc) as tc, ExitStack() as ctx:
        const = ctx.enter_context(tc.tile_pool(name="const", bufs=1))
        xp = ctx.enter_context(tc.tile_pool(name="xp", bufs=3))
        psum_bufs = 1 if mode == "epionly" else 2
        pp = ctx.enter_context(
            tc.tile_pool(name="pp", bufs=psum_bufs, space="PSUM")
        )
        sm = ctx.enter_context(tc.tile_pool(name="sm", bufs=2))

        uwt = const.tile([64, NWQ * 64], F8)
        nc.sync.dma_start(uwt, uw_d)
        uwv = uwt.rearrange("p (w i m) -> p w i m", w=NWQ, i=2)
        cs = const.tile([NPART, 4], F32)
        nc.sync.dma_start(cs, cs_d)

        xts_fixed = None
        if mode in ("nodma", "peonly", "epionly"):
            xn = ctx.enter_context(tc.tile_pool(name="xn", bufs=1))
            xts_fixed = []
            for ci, (co, cw) in enumerate(chunk_off):
                xt = xn.tile([64, cw * 1024], F8, tag=f"xtf{ci}")
                nc.sync.dma_start(
                    xt, x_d[:, co * 1024:(co + cw) * 1024]
                )
                xts_fixed.append(xt)

        def emit_matmuls(Ps):
            for ci, (co, cw) in enumerate(chunk_off):
                if xts_fixed is not None:
                    xt = xts_fixed[ci]
                else:
                    xt = xp.tile([64, cw * 1024], F8, tag="xt")
                    nc.sync.dma_start(
                        xt, x_d[:, co * 1024:(co + cw) * 1024]
                    )
                xv = xt.rearrange("p (g i r) -> p g i r", g=cw, i=2)
                for j in range(cw):
                    g = co + j
                    b, m = group_slot(g)
                    nc.tensor.matmul(
                        Ps[b],
                        uwv[:, m],
                        xv[:, j],
                        start=(g < NBANK),
                        stop=(g >= NGRP - NBANK),
                        perf_mode=mybir.MatmulPerfMode.DoubleRow,
                        skip_group_check=True,
                    )

        def emit_epilogue(Ps):
            t_t = sm.tile([NPART, NBANK, 512], BF16, tag="t_t")
            e_t = sm.tile([NPART, NBANK, 512], BF16, tag="e_t")
            fin = sm.tile([NPART, 8], F32, tag="fin")
            for b in (1, 2, 3, 0):     # bank completion order
                nc.vector.tensor_reduce(
                    fin[:, 4 + b:5 + b], Ps[b],
                    axis=mybir.AxisListType.X, op=mybir.AluOpType.max,
                )
                nc.scalar.activation(
                    t_t[:, b, :], Ps[b], mybir.ActivationFunctionType.Tanh,
                    bias=cs[:, 0:1], scale=cs[:, 1:2],
                )
                nc.scalar.activation(
                    e_t[:, b, :], t_t[:, b, :],
                    mybir.ActivationFunctionType.Exp,
                    bias=cs[:, 2:3], scale=float(CLIP),
                    accum_out=fin[:, b:b + 1],
                )
            # gpsimd (SWDGE) queue: the in-order sync queue must keep
            # streaming the next rep's x chunks; an out-DMA there would
            # stall them behind this rep's epilogue
            nc.gpsimd.dma_start(out_d, fin)

        Ps_fixed = None
        if mode == "epionly":
            Ps_fixed = [pp.tile([NPART, 512], F32, tag=f"P{b}",
                                 name=f"Pf{b}")
                        for b in range(NBANK)]
            emit_matmuls(Ps_fixed)

        for _rep in range(reps):
            if mode == "dmaonly":
                for co, cw in chunk_off:
                    xt = xp.tile([64, cw * 1024], F8, tag="xt")
                    nc.sync.dma_start(
                        xt, x_d[:, co * 1024:(co + cw) * 1024]
                    )
                continue

            if Ps_fixed is not None:
                Ps = Ps_fixed
            else:
                Ps = [pp.tile([NPART, 512], F32, tag=f"P{b}",
                              name=f"P{b}")
                      for b in range(NBANK)]
                emit_matmuls(Ps)
            if mode == "peonly":
                continue
            emit_epilogue(Ps)

        if mode in ("dmaonly", "peonly"):
            zo = sm.tile([NPART, 8], F32, tag="zo")
            nc.vector.memset(zo, 0.0)
            nc.sync.dma_start(out_d, zo)

    if dedup:
        _dedup_ldweights(nc)
    nc.compile()
    return nc


def _dedup_ldweights(nc):
    """Delete standalone InstLdweights whose weights AP is identical to
    the previous (retained) load — the PE array keeps its weights across
    matmuls, so repeats are pure overhead.  Loads carrying sync waits
    are kept (safety)."""
    removed = 0
    for blk in nc.m.functions[0].blocks:
        last_sig = None
        to_remove = []
        for inst in list(blk.instructions):
            op = str(inst.opcode)
            if op == "Ldweights":
                ap = inst.ins[0]
                sig = (getattr(ap, "offset", None), str(ap))
                si = inst.sync_info
                has_wait = si is not None and len(si.on_wait) > 0
                has_upd = si is not None and len(si.on_update) > 0
                if sig == last_sig and not has_wait and not has_upd:
                    to_remove.append(inst)
                else:
                    last_sig = sig
            elif op == "Matmult":
                pass                       # matmuls keep weights loaded
            elif op in ("EventSemaphore", "Drain"):
                pass                       # no effect on the PE array
            else:
                pass
        for inst in to_remove:
            blk.instructions.remove(inst)
            removed += 1
    return removed


# compatibility aliases for calib/test harnesses
build_program_v3 = build_program_v7
build_program_v6 = build_program_v7

_CACHE = {}


def _get_program():
    if "nc7" not in _CACHE:
        _CACHE["nc7"] = build_program_v7()
    return _CACHE["nc7"]


def _prep(output, adj_modified, W1, b1, W2, b2, prev_node):
    """Host-side scalar/vector prep shared by in-map building and combine."""
    output = np.ascontiguousarray(np.asarray(output, dtype=np.float32))
    adj = np.asarray(adj_modified, dtype=np.float32)
    W1 = np.asarray(W1, dtype=np.float64)
    b1 = np.asarray(b1, dtype=np.float64)
    W2 = np.asarray(W2, dtype=np.float64)
    b2 = np.asarray(b2, dtype=np.float64)
    pn = int(np.asarray(prev_node))

    v_i = output[pn].astype(np.float64)
    phi1 = W1 @ v_i + b1
    u = (phi1 @ W2) / np.sqrt(DH)
    cst = float(phi1 @ b2) / np.sqrt(DH)

    umax = float(np.abs(u).max())
    SC = 128.0 / umax if umax > 0 else 1.0
    u_q = (u * SC).astype(np.float32).astype(E4NP)

    idx = np.nonzero(adj > 0)[0].astype(np.int64)
    return output, u, cst, SC, u_q, idx


def make_in_maps_v7(output, adj_modified, W1, b1, W2, b2, prev_node,
                    **_ignored):
    output, u, cst, SC, u_q, idx = _prep(
        output, adj_modified, W1, b1, W2, b2, prev_node
    )
    T = idx.size
    TOT = RC * NCORES
    assert T <= TOT, f"capacity exceeded: {T} > {TOT}"

    u_qf = u_q.astype(np.float32)
    x_pad = (-64.0 * np.sign(u_qf)).astype(E4NP)

    xs = np.empty((TOT, H), dtype=E4NP)
    xs[:T] = output[idx].astype(E4NP)
    xs[T:] = x_pad[None, :]

    # one-hot weight windows: window m places u on weight column m
    # (psum partition m of bank g%4 for group g = 4m+b)
    uw = np.zeros((64, NWQ, 2, 32), dtype=E4NP)
    u2 = u_q.reshape(2, 64)                    # [i, k]
    for m in range(NWQ):
        uw[:, m, :, m] = u2.T
    uw = np.ascontiguousarray(uw.reshape(64, NWQ * 64))

    cs = np.zeros((NPART, 4), dtype=np.float32)
    cs[:, 0] = np.float32(cst)
    cs[:, 1] = np.float32(1.0 / SC)
    cs[:, 2] = np.float32(-SHIFT)

    in_maps = []
    for c in range(NCORES):
        xc = xs[c * RC:(c + 1) * RC]                # [RC, 128]
        # [g, r, i, k] -> [k, g, i, r]
        x8 = np.ascontiguousarray(
            xc.reshape(NGRP, GROUP, 2, 64).transpose(3, 0, 2, 1)
        ).reshape(64, NGRP * 1024)
        in_maps.append({"x8": x8, "uw": uw, "cs": cs})
    return in_maps


make_in_maps_v3 = make_in_maps_v7
make_in_maps_v6 = make_in_maps_v7


def combine_v7(stats, output, u, cst, SC, idx):
    """stats: [NCORES, NPART, 8] f32; cols 0..3 = per-partition z of
    bank b, cols 4..7 = per-partition raw max of bank b.  Exact host
    argmax via fp64 recheck of top groups."""
    stats = np.asarray(stats, dtype=np.float64)
    T = idx.size
    TOT = RC * NCORES
    count0 = N - T
    padcount = TOT - T

    u_qf = (u * SC).astype(np.float32).astype(E4NP).astype(np.float64)
    x_padf = (-64.0 * np.sign(u_qf)).astype(E4NP).astype(np.float64)
    pad_t = float(np.tanh(np.float32(float(x_padf @ u_qf) / SC + cst)))
    pad_contrib = padcount * np.exp(10.0 * pad_t - SHIFT)
    fake_per_core = (NBANK * NPART - NGRP) * GROUP
    fake_contrib = (NCORES * fake_per_core
                    * np.exp(10.0 * np.tanh(np.float64(np.float32(cst)))
                             - SHIFT))
    z = (float(stats[:, :, 0:4].sum()) - pad_contrib - fake_contrib
         + count0 * np.exp(-SHIFT))

    # candidate groups by quantized raw max: group g at
    # stats[core, g//4, 4 + g%4]
    g_all = np.arange(NGRP)
    smax = stats[:, g_all // NBANK, 4 + (g_all % NBANK)].reshape(-1) / SC
    gmax = float(smax.max())
    cand = np.nonzero(smax >= gmax - MARGIN)[0]

    best_attn = -np.inf
    best_row = -1
    for cg in cand:
        glo = int(cg) * GROUP                  # global padded row offset
        ghi = glo + GROUP
        if glo >= T:
            continue                           # pure padding group
        rr = idx[glo:min(ghi, T)]              # original row ids
        xr = output[rr].astype(np.float64)
        s_ex = xr @ u + cst
        attn_ex = 10.0 * np.tanh(s_ex)
        # replicate reference's (attn != 0) mask in fp32
        attn32 = (np.float32(10.0) * np.tanh(
            (xr.astype(np.float32) @ u.astype(np.float32))
            + np.float32(cst))).astype(np.float32)
        attn_ex = np.where(attn32 == 0.0, -np.inf, attn_ex)
        mx = float(attn_ex.max())
        if not np.isfinite(mx):
            continue
        tied = rr[attn_ex == mx]
        row = int(tied.min())
        if mx > best_attn or (mx == best_attn and row < best_row):
            best_attn = mx
            best_row = row

    if best_row < 0 or z <= 0:
        return np.int32(0), np.float32(0.0)
    p = np.exp(best_attn - SHIFT) / z
    return np.int32(best_row), np.float32(p)


def kernel(output, adj_modified, W1, b1, W2, b2, prev_node):
    from concourse.bass_utils import run_bass_kernel_spmd

    outf, u, cst, SC, u_q, idx = _prep(
        output, adj_modified, W1, b1, W2, b2, prev_node
    )
    if idx.size == 0:
        return np.int32(0), np.float32(0.0)

    nc = _get_program()
    in_maps = make_in_maps_v7(
        output, adj_modified, W1, b1, W2, b2, prev_node
    )
    res = run_bass_kernel_spmd(nc, in_maps, core_ids=list(range(NCORES)))
    stats = np.stack([res.results[c]["o"] for c in range(NCORES)])
    return combine_v7(stats, outf, u, cst, SC, idx)


# revision 23
# speedup vs baseline: 1.1717x; 1.0104x over previous
"""Trainium2 Bass kernel for nn_Decoder sparse-attention decode step (v6).

Math (algebraically reduced from the reference):
    phi1 = output[prev_node] @ W1.T + b1                      # [HID]
    u    = (phi1 @ W2) / sqrt(DH)                             # [H]
    cst  = (phi1 @ b2) / sqrt(DH)                             # scalar
    s[n]    = u . output[n] + cst                             # [N]
    attn[n] = 10 * adj[n] * tanh(s[n])
    w = softmax(attn); w *= (attn != 0); p = w.max(); sel = argmax(w)

v7 strategy (throughput-oriented):
  * Host drops all adj==0 rows (~50% of N): their attn is exactly 0, so
    only their count matters for the softmax denominator (added back on
    host as count0 * exp(-10) with fixed shift 10).
  * Compacted rows are quantized to fp8-e4m3 and staged PRE-TRANSPOSED
    (feature h = i*64 + k packed for DoubleRow): zero device transposes.
    One DoubleRow fp8 matmul per 512-row group accumulates scores
    straight into PSUM; group g's one-hot weight column g places it on
    psum partition g of ONE [32,512] psum bank.
  * PE instruction-count minimization: the measured PE floor here is
    ~150 ns per instruction, so group g maps to psum bank g%4 (four
    [32,512] banks) with one-hot weight column g//4 — only 7 distinct
    weight windows.  The redundant standalone LDWEIGHTS instructions
    (same weights AP as the previous load) are deleted post-Tile:
    25 matmuls need just 7 weight loads.
  * Epilogue per bank: DVE max (raw scores, per partition), ACT tanh,
    ACT exp+accum into per-partition z (bf16 activations halve ACT
    time; only the fp32 accumulator and max feed the result).  [32,8]
    stats DMA'd out on the gpsimd (SWDGE) queue so the sync DMA queue
    never stalls.  Psum one-hot zero rows contribute the known constant
    exp(10*tanh(cst)-10) per element to z; host subtracts them.
  * The argmax is done EXACTLY on the host: the top groups by quantized
    max (within a margin) are re-scored in fp64 against the original
    fp32 rows, so fp8 noise cannot flip the selection.
  * Pad rows use x_pad = -64*sign(u): tanh saturates to -1.0 exactly,
    contributing exp(-20) each to z, subtracted on host.

Per core: 25 groups x 512 rows = 12800 padded rows (fp8: 1.64 MB ->
~3.5us at ~460 GB/s/core measured sync-queue DMA).
"""

from contextlib import ExitStack

import numpy as np
import ml_dtypes

import concourse.bass as bass
import concourse.bacc as bacc
import concourse.tile as tile
from concourse import mybir

F32 = mybir.dt.float32
F8 = mybir.dt.float8e4
E4NP = ml_dtypes.float8_e4m3

N = 200000
H = 128
HID = 512
DH = 512.0
CLIP = 10.0
NCORES = 8
GROUP = 512
NGRP = 25                      # groups per core
RC = NGRP * GROUP              # 12800 padded rows per core
NBANK = 4                      # psum banks; group g -> bank g%4
NWQ = (NGRP + NBANK - 1) // NBANK   # 7 weight windows (column g//4)
NPART = 32                     # partitions per psum bank
SHIFT = 10.0                   # fixed softmax shift (attn <= 10 always)
MARGIN = 0.5                   # host recheck margin, in s units
CHGS = (5, 5, 5, 5, 5)         # DMA chunks in groups


def group_slot(g):
    """(bank, partition) of group g."""
    return g % NBANK, g // NBANK


def build_program_v7(reps=1, mode="full", dedup=True, act16=True,
                     fd=512):
    """mode: 'full' | 'dmaonly' (only the x DMAs per rep) |
    'nodma' (DMA once, repeat compute on stale tiles) |
    'peonly' (DMA once, repeat matmuls only) |
    'epionly' (DMA + matmuls once, repeat the epilogue only).
    dedup: delete standalone LDWEIGHTS whose weights AP repeats.
    act16: bf16 activation outputs (z accum stays fp32)."""
    nc = bacc.Bacc(
        "TRN2", target_bir_lowering=False, debug=False, num_devices=NCORES
    )

    BF16 = mybir.dt.bfloat16 if act16 else F32

    x_d = nc.dram_tensor(
        "x8", [64, NGRP * 1024], F8, kind="ExternalInput"
    ).ap()
    uw_d = nc.dram_tensor(
        "uw", [64, NWQ * 64], F8, kind="ExternalInput"
    ).ap()
    cs_d = nc.dram_tensor("cs", [NPART, 4], F32, kind="ExternalInput").ap()
    out_d = nc.dram_tensor("o", [NPART, 8], F32, kind="ExternalOutput").ap()

    chunk_off = []
    off = 0
    for w in CHGS:
        chunk_off.append((off, w))
        off += w
    assert off == NGRP

    with tile.TileContext(nc) as tc, ExitStack() as ctx:
        const = ctx.enter_context(tc.tile_pool(name="const", bufs=1))
        xp = ctx.enter_context(tc.tile_pool(name="xp", bufs=6))
        psum_bufs = 1 if mode == "epionly" else 2
        pp = ctx.enter_context(
            tc.tile_pool(name="pp", bufs=psum_bufs, space="PSUM")
        )
        sm = ctx.enter_context(tc.tile_pool(name="sm", bufs=2))

        uwt = const.tile([64, NWQ * 64], F8)
        nc.sync.dma_start(uwt, uw_d)
        uwv = uwt.rearrange("p (w i m) -> p w i m", w=NWQ, i=2)
        cs = const.tile([NPART, 4], F32)
        nc.sync.dma_start(cs, cs_d)

        xts_fixed = None
        if mode in ("nodma", "peonly", "epionly"):
            xn = ctx.enter_context(tc.tile_pool(name="xn", bufs=1))
            xts_fixed = []
            for ci, (co, cw) in enumerate(chunk_off):
                xt = xn.tile([64, cw * 1024], F8, tag=f"xtf{ci}")
                nc.sync.dma_start(
                    xt, x_d[:, co * 1024:(co + cw) * 1024]
                )
                xts_fixed.append(xt)

        def emit_matmuls(P):
            for ci, (co, cw) in enumerate(chunk_off):
                if xts_fixed is not None:
                    xt = xts_fixed[ci]
                else:
                    xt = xp.tile([64, cw * 1024], F8, tag="xt")
                    nc.sync.dma_start(
                        xt, x_d[:, co * 1024:(co + cw) * 1024]
                    )
                xv = xt.rearrange("p (g i h) -> p g i h", g=cw, i=2)
                for j in range(cw):
                    g = co + j
                    b, m = group_slot(g)
                    for f0 in range(0, 512, fd):
                        nc.tensor.matmul(
                            P[:, b * 512 + f0:b * 512 + f0 + fd],
                            uwv[:, m],
                            xv[:, j, :, f0:f0 + fd],
                            start=(g < NBANK),
                            stop=(g >= NGRP - NBANK),
                            perf_mode=mybir.MatmulPerfMode.DoubleRow,
                            skip_group_check=True,
                        )

        def emit_epilogue(P):
            t_t = sm.tile([NPART, NBANK * 512], BF16, tag="t_t")
            e_t = sm.tile([NPART, NBANK * 512], BF16, tag="e_t")
            fin = sm.tile([NPART, 8], F32, tag="fin")
            Pv = P.rearrange("p (b r) -> p b r", b=NBANK)
            nc.vector.tensor_reduce(
                fin[:, 4:8], Pv,
                axis=mybir.AxisListType.X, op=mybir.AluOpType.max,
            )
            nc.scalar.activation(
                t_t, P, mybir.ActivationFunctionType.Tanh,
                bias=cs[:, 0:1], scale=cs[:, 1:2],
            )
            nc.scalar.activation(
                e_t, t_t, mybir.ActivationFunctionType.Exp,
                bias=cs[:, 2:3], scale=float(CLIP),
                accum_out=fin[:, 0:1],
            )
            # gpsimd (SWDGE) queue: the in-order sync queue must keep
            # streaming the next rep's x chunks; an out-DMA there would
            # stall them behind this rep's epilogue
            nc.gpsimd.dma_start(out_d, fin)

        P_fixed = None
        if mode == "epionly":
            P_fixed = pp.tile([NPART, NBANK * 512], F32, tag="P")
            emit_matmuls(P_fixed)

        for _rep in range(reps):
            if mode == "dmaonly":
                for co, cw in chunk_off:
                    xt = xp.tile([64, cw * 1024], F8, tag="xt")
                    nc.sync.dma_start(
                        xt, x_d[:, co * 1024:(co + cw) * 1024]
                    )
                continue

            if P_fixed is not None:
                P = P_fixed
            else:
                P = pp.tile([NPART, NBANK * 512], F32, tag="P")
                emit_matmuls(P)
            if mode in ("peonly", "noepi"):
                continue
            emit_epilogue(P)

        if mode in ("dmaonly", "peonly", "noepi"):
            zo = sm.tile([NPART, 8], F32, tag="zo")
            nc.vector.memset(zo, 0.0)
            nc.sync.dma_start(out_d, zo)

    if dedup:
        _dedup_ldweights(nc)
    nc.compile()
    return nc


def _dedup_ldweights(nc):
    """Delete standalone InstLdweights whose weights AP is identical to
    the previous (retained) load — the PE array keeps its weights across
    matmuls, so repeats are pure overhead.  Loads carrying sync waits
    are kept (safety)."""
    removed = 0
    for blk in nc.m.functions[0].blocks:
        last_sig = None
        to_remove = []
        for inst in list(blk.instructions):
            op = str(inst.opcode)
            if op == "Ldweights":
                ap = inst.ins[0]
                sig = (getattr(ap, "offset", None), str(ap))
                si = inst.sync_info
                has_wait = si is not None and len(si.on_wait) > 0
                has_upd = si is not None and len(si.on_update) > 0
                if sig == last_sig and not has_wait and not has_upd:
                    to_remove.append(inst)
                else:
                    last_sig = sig
            elif op == "Matmult":
                pass                       # matmuls keep weights loaded
            elif op in ("EventSemaphore", "Drain"):
                pass                       # no effect on the PE array
            else:
                pass
        for inst in to_remove:
            blk.instructions.remove(inst)
            removed += 1
    return removed


# compatibility aliases for calib/test harnesses
build_program_v3 = build_program_v7
build_program_v6 = build_program_v7

_CACHE = {}


def _get_program():
    if "nc7" not in _CACHE:
        _CACHE["nc7"] = build_program_v7()
    return _CACHE["nc7"]


def _prep(output, adj_modified, W1, b1, W2, b2, prev_node):
    """Host-side scalar/vector prep shared by in-map building and combine."""
    output = np.ascontiguousarray(np.asarray(output, dtype=np.float32))
    adj = np.asarray(adj_modified, dtype=np.float32)
    W1 = np.asarray(W1, dtype=np.float64)
    b1 = np.asarray(b1, dtype=np.float64)
    W2 = np.asarray(W2, dtype=np.float64)
    b2 = np.asarray(b2, dtype=np.float64)
    pn = int(np.asarray(prev_node))

    v_i = output[pn].astype(np.float64)
    phi1 = W1 @ v_i + b1
    u = (phi1 @ W2) / np.sqrt(DH)
    cst = float(phi1 @ b2) / np.sqrt(DH)

    umax = float(np.abs(u).max())
    SC = 128.0 / umax if umax > 0 else 1.0
    u_q = (u * SC).astype(np.float32).astype(E4NP)

    idx = np.nonzero(adj > 0)[0].astype(np.int64)
    return output, u, cst, SC, u_q, idx


def make_in_maps_v7(output, adj_modified, W1, b1, W2, b2, prev_node,
                    **_ignored):
    output, u, cst, SC, u_q, idx = _prep(
        output, adj_modified, W1, b1, W2, b2, prev_node
    )
    T = idx.size
    TOT = RC * NCORES
    assert T <= TOT, f"capacity exceeded: {T} > {TOT}"

    u_qf = u_q.astype(np.float32)
    x_pad = (-64.0 * np.sign(u_qf)).astype(E4NP)

    xs = np.empty((TOT, H), dtype=E4NP)
    xs[:T] = output[idx].astype(E4NP)
    xs[T:] = x_pad[None, :]

    # one-hot weight windows: window m places u on weight column m
    # (psum partition m of bank g%4 for group g = 4m+b)
    uw = np.zeros((64, NWQ, 2, 32), dtype=E4NP)
    u2 = u_q.reshape(2, 64)                    # [i, k]
    for m in range(NWQ):
        uw[:, m, :, m] = u2.T
    uw = np.ascontiguousarray(uw.reshape(64, NWQ * 64))

    cs = np.zeros((NPART, 4), dtype=np.float32)
    cs[:, 0] = np.float32(cst)
    cs[:, 1] = np.float32(1.0 / SC)
    cs[:, 2] = np.float32(-SHIFT)

    in_maps = []
    for c in range(NCORES):
        xc = xs[c * RC:(c + 1) * RC]                # [RC, 128]
        # [g, r, i, k] -> [k, g, i, r]
        x8 = np.ascontiguousarray(
            xc.reshape(NGRP, GROUP, 2, 64).transpose(3, 0, 2, 1)
        ).reshape(64, NGRP * 1024)
        in_maps.append({"x8": x8, "uw": uw, "cs": cs})
    return in_maps


make_in_maps_v3 = make_in_maps_v7
make_in_maps_v6 = make_in_maps_v7


def combine_v7(stats, output, u, cst, SC, idx):
    """stats: [NCORES, NPART, 8] f32; cols 0..3 = per-partition z of
    bank b, cols 4..7 = per-partition raw max of bank b.  Exact host
    argmax via fp64 recheck of top groups."""
    stats = np.asarray(stats, dtype=np.float64)
    T = idx.size
    TOT = RC * NCORES
    count0 = N - T
    padcount = TOT - T

    u_qf = (u * SC).astype(np.float32).astype(E4NP).astype(np.float64)
    x_padf = (-64.0 * np.sign(u_qf)).astype(E4NP).astype(np.float64)
    pad_t = float(np.tanh(np.float32(float(x_padf @ u_qf) / SC + cst)))
    pad_contrib = padcount * np.exp(10.0 * pad_t - SHIFT)
    fake_per_core = (NBANK * NPART - NGRP) * GROUP
    fake_contrib = (NCORES * fake_per_core
                    * np.exp(10.0 * np.tanh(np.float64(np.float32(cst)))
                             - SHIFT))
    z = (float(stats[:, :, 0].sum()) - pad_contrib - fake_contrib
         + count0 * np.exp(-SHIFT))

    # candidate groups by quantized raw max: group g at
    # stats[core, g//4, 4 + g%4]
    g_all = np.arange(NGRP)
    smax = stats[:, g_all // NBANK, 4 + (g_all % NBANK)].reshape(-1) / SC
    gmax = float(smax.max())
    cand = np.nonzero(smax >= gmax - MARGIN)[0]

    best_attn = -np.inf
    best_row = -1
    for cg in cand:
        glo = int(cg) * GROUP                  # global padded row offset
        ghi = glo + GROUP
        if glo >= T:
            continue                           # pure padding group
        rr = idx[glo:min(ghi, T)]              # original row ids
        xr = output[rr].astype(np.float64)
        s_ex = xr @ u + cst
        attn_ex = 10.0 * np.tanh(s_ex)
        # replicate reference's (attn != 0) mask in fp32
        attn32 = (np.float32(10.0) * np.tanh(
            (xr.astype(np.float32) @ u.astype(np.float32))
            + np.float32(cst))).astype(np.float32)
        attn_ex = np.where(attn32 == 0.0, -np.inf, attn_ex)
        mx = float(attn_ex.max())
        if not np.isfinite(mx):
            continue
        tied = rr[attn_ex == mx]
        row = int(tied.min())
        if mx > best_attn or (mx == best_attn and row < best_row):
            best_attn = mx
            best_row = row

    if best_row < 0 or z <= 0:
        return np.int32(0), np.float32(0.0)
    p = np.exp(best_attn - SHIFT) / z
    return np.int32(best_row), np.float32(p)


def kernel(output, adj_modified, W1, b1, W2, b2, prev_node):
    from concourse.bass_utils import run_bass_kernel_spmd

    outf, u, cst, SC, u_q, idx = _prep(
        output, adj_modified, W1, b1, W2, b2, prev_node
    )
    if idx.size == 0:
        return np.int32(0), np.float32(0.0)

    nc = _get_program()
    in_maps = make_in_maps_v7(
        output, adj_modified, W1, b1, W2, b2, prev_node
    )
    res = run_bass_kernel_spmd(nc, in_maps, core_ids=list(range(NCORES)))
    stats = np.stack([res.results[c]["o"] for c in range(NCORES)])
    return combine_v7(stats, outf, u, cst, SC, idx)
